# revision 9
# baseline (speedup 1.0000x reference)
"""AffinityHead Trainium2 kernel (v4: chunked staging + rebalanced affinity).

Reference computation:
  f = ELU(concat(w83@conv4, w84@conv5, w85@conv6))   (1x1 convs, per pixel)
  x = ELU(w9 @ f)                                     [B, 448, 56, 56]
  aff[b,d,p] = exp(-mean_c |x[c, to(d,p)] - x[c, from(p)]|)   [B, 34, 2496]

Sharding: 8 cores = 4 images x 2 row-halves. Each core handles 26 from-rows
(+4 halo rows) = 30 rows of one image; SPMD identical program.

v4 design (measured facts from microbench, overturning v3 assumptions):
- ONE SWDGE cast-DMA is split across all 16 DMA queues by the runtime at
  the same ~205GB/s write-side ceiling as 16 small DMAs. Issue cost is
  ~950ns PER INSTRUCTION regardless of size -> stage with 6 DMAs per slab
  (c5, c4, c6 in 4 chunks of 8 ktiles) instead of 22. gpsimd issue load
  drops 95us -> ~25us, freeing Pool for late-band subtracts.
- PE matmuls run at pump speed with LDWEIGHTS fully hidden (s2s 203ns for
  480-col 1-row reduce; strided rhs legal at full speed; ldweights=False
  chain verified bit-exact). No group-folding needed; 4 matmuls per offset
  cost ~pump only. PE total ~ conv 64us + affinity 62us pump.
- DVE TT(sub/add) is port-bound at ~0.54ns/elem (3 streams / 2 ports) in
  ALL access patterns (strided == contiguous, alignment irrelevant) ->
  the v3 xo odd-shift copy was useless; subtract reads xg directly for
  any (dy,dx). int16 mask-abs (TS, 2 streams) ~0.30ns/elem. STT-abs 1x.
  ACT Abs ~0.93ns/elem. Pool TT ~1.75ns/elem.
- DVE is the wall: all subs+masks = 147us. Offload ~1/3 of abs to ACT and
  a few late subs to Pool (idle after staging issues).
- slabs {420,420,512,328}: last x chunk lands earlier; bands
  (0,11)@slab1, (11,9)@slab2, (20,6)@slab3 -> 6-row tail.

Stack constraints (kept from v3 + new):
- build on bacc.Bacc and call nc.finalize().
- matmul/AP base partition must be 0, 32, or 64.
- abs_max ALU op does not exist in this walrus; scalar_tensor_tensor
  lowers to TensorScalarPtr which Pool rejects (no Pool abs).
- only gpsimd can issue casting DMAs (SWDGE); Pool band ops must be
  enqueued after ALL staging issues (in-order queue).
- PSUM bank = 512 f32 free; 8 banks: f85x2 + f84 + f83 + x + 3 aff.
"""
import numpy as np
from contextlib import ExitStack

import concourse.bass as bass
from concourse import bacc
import concourse.mybir as mybir
import concourse.tile as tile
from concourse.bass_utils import run_bass_kernel_spmd

RAD = 5
W = 56
ROWS = 30            # rows of x per core (26 from + 4 halo)
FROM_ROWS = 26
NPX = ROWS * W       # 1680
NPAIR = FROM_ROWS * 48   # 1248
C = 448
N_CORES = 8

F32 = mybir.dt.float32
BF16 = mybir.dt.bfloat16
I16 = mybir.dt.int16


def _offsets():
    out = []
    for x in range(1, RAD):
        out.append((0, x))
    for y in range(1, RAD):
        for x in range(-RAD + 1, RAD):
            if x * x + y * y < RAD * RAD:
                out.append((y, x))
    return out


OFFS = _offsets()            # 34 (dy, dx), matching reference search_dist order
assert len(OFFS) == 34

# w9 contraction split aligned to feature-group boundaries (f83|f84|f85a|f85b)
KSPLIT = [(0, 64), (64, 128), (192, 128), (320, 128)]
# x output channel groups: 4 groups of <=128 (padded to 128 in storage)
MSPLIT = [(0, 128), (128, 128), (256, 128), (384, 64)]

# pixel slabs (start, width); widths <= 512 (PSUM bank) and sum to NPX
SLABS = [(0, 420), (420, 420), (840, 512), (1352, 328)]
NSLAB = len(SLABS)

# affinity bands: (from_row0, nrows, emit_after_slab_index)
BANDS = [(0, 10, 1), (10, 10, 2), (20, 6, 3)]
for _r0, _nr, _si in BANDS:
    _need = (_r0 + _nr + 4) * W
    _s0, _w = SLABS[_si]
    assert _need <= _s0 + _w, (_r0, _nr, _si)
assert sum(b[1] for b in BANDS) == FROM_ROWS

# per-(band, offset) engine assignment:
# sub: 'v' = DVE, 'p' = Pool (only bands >=1: Pool queue drains staging
# issues first). abs: 'v' = DVE int16 mask, 'a' = ACT Abs.
# pool-dedicated triples: the LAST triples of each band run their subtracts
# on Pool (idle after staging issues) with a dedicated PSUM bank + dt pools
# so the slow pool stream never blocks the DVE-stream rings or queue heads.
POOL_T = [set(), set(), set()]
SUB_E = [['p' if d // 3 in POOL_T[b] else 'v' for d in range(34)]
         for b in range(3)]
_ACT_MOD = [3, 3, 3]   # band0 lighter ACT share (elu overlap during staging)
ABS_E = [['a' if (d % _ACT_MOD[b] == 1 and d // 3 not in POOL_T[b]) else 'v'
          for d in range(34)] for b in range(3)]

# pump schedule: triples pumped per conv section
PUMP_CHUNK = 3    # after each c6 chunk (4 per slab)
PUMP_F84 = 1
PUMP_F83 = 1
PUMP_X = 1        # after each of 4 x m-tiles


def _emit(ctx: ExitStack, tc: "tile.TileContext", io: dict):
    nc = tc.nc
    c6, c5, c4 = io["c6"], io["c5"], io["c4"]
    out_d = io["out"]

    persist = ctx.enter_context(tc.tile_pool(name="persist", bufs=1))
    stage6 = ctx.enter_context(tc.tile_pool(name="stage6", bufs=7))
    stage5 = ctx.enter_context(tc.tile_pool(name="stage5", bufs=2))
    stage4 = ctx.enter_context(tc.tile_pool(name="stage4", bufs=2))
    fpool = ctx.enter_context(tc.tile_pool(name="fpool", bufs=3))
    tpool = ctx.enter_context(tc.tile_pool(name="tmp", bufs=4))
    dpool = ctx.enter_context(tc.tile_pool(name="dtv", bufs=8))
    d2pool = ctx.enter_context(tc.tile_pool(name="dt2", bufs=4))
    pdt = ctx.enter_context(tc.tile_pool(name="pdt", bufs=3))
    pda = ctx.enter_context(tc.tile_pool(name="pda", bufs=2))
    apool = ctx.enter_context(tc.tile_pool(name="aff", bufs=4))
    psF = ctx.enter_context(tc.tile_pool(name="psF", bufs=1, space="PSUM"))
    psX = ctx.enter_context(tc.tile_pool(name="psX", bufs=1, space="PSUM"))
    psA = ctx.enter_context(tc.tile_pool(name="psA", bufs=2, space="PSUM"))
    psP = ctx.enter_context(tc.tile_pool(name="psP", bufs=1, space="PSUM"))

    # ---- weights into SBUF: ONE packed bf16 DMA each ----
    wcs = persist.tile([128, 9472], BF16, name="wcs")
    nc.sync.dma_start(wcs[:], io["wc"][:])
    w9cs = persist.tile([128, 4, 448], BF16, name="w9cs")
    nc.sync.dma_start(w9cs[:], io["w9c"][:].rearrange("p (k m) -> p k m", k=4))

    def w85_sl(kt, m):
        base = kt * 256 + m * 128
        return wcs[:, base:base + 128]

    def w84_sl(kt):
        return wcs[:, 8192 + kt * 128:8192 + (kt + 1) * 128]

    def w83_sl(kt):
        return wcs[:, 9216 + kt * 64:9216 + (kt + 1) * 64]

    ones = persist.tile([128, 1], BF16, name="ones")
    nc.vector.memset(ones[:], 1.0)

    # ---- x storage (bf16, 4x128 padded groups) ----
    xg = persist.tile([128, 4, NPX], BF16, name="xg", tag="xg")
    # zero the pad rows of group 3 (channels 448..511); elu writes 0:64 only
    nc.vector.memset(xg[64:128, 3, :], 0.0)

    # ---- ELU helper: out = max(p, exp(min(p,0)) - 1), p in PSUM.
    def elu(psrc, dst, pn, fn):
        r = tpool.tile([pn, 512], BF16, tag="elu_m", name="elu_m")
        nc.scalar.activation(out=r[:, :fn], in_=psrc, scale=-1.0,
                             func=mybir.ActivationFunctionType.Relu)
        e = tpool.tile([pn, 512], BF16, tag="elu_e", name="elu_e")
        nc.scalar.activation(out=e[:, :fn], in_=r[:, :fn], scale=-1.0,
                             func=mybir.ActivationFunctionType.Exp)
        nc.vector.scalar_tensor_tensor(
            out=dst, in0=e[:, :fn], scalar=-1.0, in1=psrc,
            op0=mybir.AluOpType.add, op1=mybir.AluOpType.max)

    # ---- conv input staging: cast-DMA (fp32 HBM -> bf16 SBUF), 6 DMAs
    # per slab (c5, c4, c6 as 4 chunks of 8 ktiles). A single SWDGE DMA is
    # runtime-split across all 16 queues at the same aggregate bandwidth;
    # issue cost is per-instruction, so fewer+bigger wins.
    v6 = c6[:].rearrange("(k p) n -> p k n", p=128)
    v5 = c5[:].rearrange("(k p) n -> p k n", p=128)
    v4 = c4[:].rearrange("(k p) n -> p k n", p=128)
    cslab = []
    for s0, w in SLABS:
        t5 = stage5.tile([128, 8, 512], BF16, tag="c5", name="c5")
        nc.gpsimd.dma_start(t5[:, :, :w], v5[:, :, s0:s0 + w])
        t4 = stage4.tile([128, 4, 512], BF16, tag="c4", name="c4")
        nc.gpsimd.dma_start(t4[:, :, :w], v4[:, :, s0:s0 + w])
        t6 = []
        for ck in range(4):
            t = stage6.tile([128, 8, 512], BF16, tag="c6", name="c6")
            nc.gpsimd.dma_start(t[:, :, :w], v6[:, ck * 8:(ck + 1) * 8, s0:s0 + w])
            t6.append(t)
        cslab.append({"c6": t6, "c5": t5, "c4": t4})

    xg_r = xg[:].rearrange("p g (r c) -> p g r c", c=W)

    # Band triples are emitted as a generator and pumped between conv
    # sections so band work interleaves into engine queues in dep order.
    def emit_band(bi, r0, nr):
        npair = nr * 48
        for t3 in range(12):
            k = min(3, 34 - t3 * 3)
            is_pool = t3 in POOL_T[bi]
            if is_pool:
                pst = psP.tile([128, 512], F32, tag="pstp", name="pstp")
            else:
                pst = psA.tile([128, 512], F32, tag="pst", name="pst")
            arow = apool.tile([65, 480], F32, tag="arow", name="arow")
            for j in range(k):
                d_idx = t3 * 3 + j
                dy, dx = OFFS[d_idx]
                to_view = xg_r[:, :, r0 + dy:r0 + dy + nr, 4 + dx:52 + dx]
                from_view = xg_r[:, :, r0:r0 + nr, 4:52]
                if is_pool:
                    dt = pdt.tile([128, 4, 10, 48], BF16, tag="pdt", name="pdt")
                else:
                    dt = dpool.tile([128, 4, 10, 48], BF16, tag="dt", name="dt")
                dslice = dt[:, :, 0:nr, :]
                if SUB_E[bi][d_idx] == 'p':
                    nc.gpsimd.tensor_tensor(out=dslice, in0=to_view,
                                            in1=from_view,
                                            op=mybir.AluOpType.subtract)
                else:
                    nc.vector.tensor_tensor(out=dslice, in0=to_view,
                                            in1=from_view,
                                            op=mybir.AluOpType.subtract)
                if is_pool:
                    a = pda.tile([128, 4, 10, 48], BF16, tag="pda", name="pda")
                else:
                    a = d2pool.tile([128, 4, 10, 48], BF16, tag="da", name="da")
                aslice = a[:, :, 0:nr, :]
                if ABS_E[bi][d_idx] == 'a':
                    nc.scalar.activation(out=aslice, in_=dslice,
                                         func=mybir.ActivationFunctionType.Abs)
                else:
                    nc.vector.tensor_scalar(
                        out=aslice.bitcast(I16), in0=dslice.bitcast(I16),
                        scalar1=32767, scalar2=None,
                        op0=mybir.AluOpType.bitwise_and)
                for g in range(4):
                    nc.tensor.matmul(
                        pst[32 * j:32 * j + 1, :npair], ones[:],
                        a[:, g, 0:nr, :],
                        start=(g == 0), stop=(g == 3))
            pn = 32 * (k - 1) + 1
            nc.scalar.activation(out=arow[0:pn, :npair], in_=pst[0:pn, :npair],
                                 func=mybir.ActivationFunctionType.Exp,
                                 scale=-1.0 / C)
            for j in range(k):
                d_out = t3 * 3 + j
                nc.sync.dma_start(
                    out_d[d_out:d_out + 1, r0 * 48:(r0 + nr) * 48],
                    arow[32 * j:32 * j + 1, :npair])
            yield

    band_q = list(BANDS)
    active = []

    def pump(n):
        while n > 0 and active:
            try:
                next(active[0])
                n -= 1
            except StopIteration:
                active.pop(0)

    for s in range(NSLAB):
        s0, w = SLABS[s]
        hs = cslab[s]

        def load6(kt):
            return hs["c6"][kt // 8][:, kt % 8, :w]

        # f85: 2 M-tiles of 128 out-ch
        f85p = [psF.tile([128, 512], F32, tag=f"f85{m}", name=f"f85p{m}")
                for m in range(2)]
        for kt in range(32):
            rhs = load6(kt)
            for m in range(2):
                nc.tensor.matmul(
                    f85p[m][:, :w], w85_sl(kt, m),
                    rhs, start=(kt == 0), stop=(kt == 31))
            if kt % 8 == 7:
                pump(PUMP_CHUNK)
        f84p = psF.tile([128, 512], F32, tag="f84", name="f84p")
        for kt in range(8):
            nc.tensor.matmul(f84p[:, :w], w84_sl(kt), hs["c5"][:, kt, :w],
                             start=(kt == 0), stop=(kt == 7))
        pump(PUMP_F84)
        f83p = psF.tile([64, 512], F32, tag="f83", name="f83p")
        for kt in range(4):
            nc.tensor.matmul(f83p[:, :w], w83_sl(kt), hs["c4"][:, kt, :w],
                             start=(kt == 0), stop=(kt == 3))
        pump(PUMP_F83)

        # ELU f -> sbuf k-group tiles (64/128/128/128 partitions)
        fk = [fpool.tile([kn, 512], BF16, tag=f"fk{i}", name=f"fk{i}")
              for i, (k0, kn) in enumerate(KSPLIT)]
        elu(f83p[:, :w], fk[0][:, :w], 64, w)
        elu(f84p[:, :w], fk[1][:, :w], 128, w)
        elu(f85p[0][:, :w], fk[2][:, :w], 128, w)
        elu(f85p[1][:, :w], fk[3][:, :w], 128, w)

        # x = ELU(w9 @ f): M-tiles sequential to cap PSUM use
        sl = slice(s0, s0 + w)
        for mt, (m0, mn) in enumerate(MSPLIT):
            xp = psX.tile([mn, 512], F32, tag="xp", name="xp")
            for kt in range(4):
                nc.tensor.matmul(xp[:, :w], w9cs[0:KSPLIT[kt][1], kt, m0:m0 + mn],
                                 fk[kt][:, :w], start=(kt == 0), stop=(kt == 3))
            elu(xp[:, :w], xg[0:mn, mt, sl], mn, w)
            pump(PUMP_X)

        # activate any bands whose x pixels are now all written
        while band_q and band_q[0][2] == s:
            r0, nr, _ = band_q.pop(0)
            active.append(emit_band(len(BANDS) - len(band_q) - 1, r0, nr))

    # drain remaining band work
    pump(10 ** 6)


_NC_CACHE = {}
LAST_RESULT = None


def _build_nc():
    if "nc" in _NC_CACHE:
        return _NC_CACHE["nc"]
    nc = bacc.Bacc()
    io = {
        "c6": nc.declare_dram_parameter("c6", [4096, NPX], F32, isOutput=False),
        "c5": nc.declare_dram_parameter("c5", [1024, NPX], F32, isOutput=False),
        "c4": nc.declare_dram_parameter("c4", [512, NPX], F32, isOutput=False),
        "wc": nc.declare_dram_parameter("wc", [128, 9472], BF16, isOutput=False),
        "w9c": nc.declare_dram_parameter("w9c", [128, 4 * 448], BF16, isOutput=False),
        "out": nc.declare_dram_parameter("out", [34, NPAIR], F32, isOutput=True),
    }
    with tile.TileContext(nc) as tc:
        with ExitStack() as ctx:
            _emit(ctx, tc, io)
    nc.finalize()
    _NC_CACHE["nc"] = nc
    return nc


def _expected_indices():
    full = np.reshape(np.arange(0, 56 * 56, dtype=np.int64), (56, 56))
    ind_from = np.reshape(full[:-4, 4:-4], [-1])
    tos = []
    for dy, dx in OFFS:
        tos.append(np.reshape(full[dy:dy + 52, 4 + dx:4 + dx + 48], [-1]))
    return ind_from, np.concatenate(tos, axis=0)


def _maybe_install_trace_hook():
    import os
    if not os.environ.get("BASS_TRACE"):
        return
    import sys
    import types
    try:
        import antenv.axon_hooks  # noqa: F401
        return
    except ImportError:
        pass
    try:
        from trn_agent_boot.trn_boot import _ntff_profile_via_ctypes
        hook = _ntff_profile_via_ctypes('/opt/axon/libaxon_pjrt.so')
    except Exception:
        hook = None
    import antenv
    mod = types.ModuleType("antenv.axon_hooks")
    mod.get_axon_ntff_profile_hook = lambda: hook
    mod.set_axon_ntff_profile_hook = lambda h: None
    sys.modules["antenv.axon_hooks"] = mod
    antenv.axon_hooks = mod


def kernel(conv4, conv5, conv6, w83, w84, w85, w9, ind_from, ind_to):
    import ml_dtypes
    conv4 = np.asarray(conv4, dtype=np.float32)
    conv5 = np.asarray(conv5, dtype=np.float32)
    conv6 = np.asarray(conv6, dtype=np.float32)
    ef, et = _expected_indices()
    assert np.array_equal(np.asarray(ind_from), ef), "unexpected ind_from"
    assert np.array_equal(np.asarray(ind_to), et), "unexpected ind_to"

    def warrange(w, ktiles):
        # w [M, K] -> lhsT tiles layout [128, ktiles*M]
        wt = np.asarray(w, np.float32).T            # [K, M]
        K, M = wt.shape
        return np.ascontiguousarray(
            wt.reshape(ktiles, 128, M).transpose(1, 0, 2).reshape(128, ktiles * M))

    wc = np.ascontiguousarray(np.concatenate(
        [warrange(w85, 32), warrange(w84, 8), warrange(w83, 4)],
        axis=1)).astype(ml_dtypes.bfloat16)
    w9t_f = np.asarray(w9, np.float32).T          # [448 in, 448 out]
    w9c = np.zeros((128, 4, 448), np.float32)
    ks = [(0, 64), (64, 128), (192, 128), (320, 128)]
    for i, (k0, kn) in enumerate(ks):
        w9c[0:kn, i, :] = w9t_f[k0:k0 + kn, :]
    w9c = np.ascontiguousarray(w9c.reshape(128, 4 * 448)).astype(
        ml_dtypes.bfloat16)

    in_maps = []
    for core in range(N_CORES):
        b, half = core // 2, core % 2
        r0 = 0 if half == 0 else 26
        in_maps.append({
            "c6": np.ascontiguousarray(
                conv6[b, :, r0:r0 + ROWS, :].reshape(4096, NPX)),
            "c5": np.ascontiguousarray(
                conv5[b, :, r0:r0 + ROWS, :].reshape(1024, NPX)),
            "c4": np.ascontiguousarray(
                conv4[b, :, r0:r0 + ROWS, :].reshape(512, NPX)),
            "wc": wc, "w9c": w9c,
        })

    _maybe_install_trace_hook()
    nc = _build_nc()
    res = run_bass_kernel_spmd(nc, in_maps, list(range(N_CORES)))
    global LAST_RESULT
    LAST_RESULT = res

    aff = np.empty((4, 34, 2496), np.float32)
    for core in range(N_CORES):
        b, half = core // 2, core % 2
        aff[b, :, half * NPAIR:(half + 1) * NPAIR] = res.results[core]["out"]
    return aff


# revision 10
# speedup vs baseline: 1.1600x; 1.1600x over previous
"""AffinityHead Trainium2 kernel (v4: chunked staging + rebalanced affinity).

Reference computation:
  f = ELU(concat(w83@conv4, w84@conv5, w85@conv6))   (1x1 convs, per pixel)
  x = ELU(w9 @ f)                                     [B, 448, 56, 56]
  aff[b,d,p] = exp(-mean_c |x[c, to(d,p)] - x[c, from(p)]|)   [B, 34, 2496]

Sharding: 8 cores = 4 images x 2 row-halves. Each core handles 26 from-rows
(+4 halo rows) = 30 rows of one image; SPMD identical program.

v4 design (measured facts from microbench, overturning v3 assumptions):
- ONE SWDGE cast-DMA is split across all 16 DMA queues by the runtime at
  the same ~205GB/s write-side ceiling as 16 small DMAs. Issue cost is
  ~950ns PER INSTRUCTION regardless of size -> stage with 6 DMAs per slab
  (c5, c4, c6 in 4 chunks of 8 ktiles) instead of 22. gpsimd issue load
  drops 95us -> ~25us, freeing Pool for late-band subtracts.
- PE matmuls run at pump speed with LDWEIGHTS fully hidden (s2s 203ns for
  480-col 1-row reduce; strided rhs legal at full speed; ldweights=False
  chain verified bit-exact). No group-folding needed; 4 matmuls per offset
  cost ~pump only. PE total ~ conv 64us + affinity 62us pump.
- DVE TT(sub/add) is port-bound at ~0.54ns/elem (3 streams / 2 ports) in
  ALL access patterns (strided == contiguous, alignment irrelevant) ->
  the v3 xo odd-shift copy was useless; subtract reads xg directly for
  any (dy,dx). int16 mask-abs (TS, 2 streams) ~0.30ns/elem. STT-abs 1x.
  ACT Abs ~0.93ns/elem. Pool TT ~1.75ns/elem.
- DVE is the wall: all subs+masks = 147us. Offload ~1/3 of abs to ACT and
  a few late subs to Pool (idle after staging issues).
- slabs {420,420,512,328}: last x chunk lands earlier; bands
  (0,11)@slab1, (11,9)@slab2, (20,6)@slab3 -> 6-row tail.

Stack constraints (kept from v3 + new):
- build on bacc.Bacc and call nc.finalize().
- matmul/AP base partition must be 0, 32, or 64.
- abs_max ALU op does not exist in this walrus; scalar_tensor_tensor
  lowers to TensorScalarPtr which Pool rejects (no Pool abs).
- only gpsimd can issue casting DMAs (SWDGE); Pool band ops must be
  enqueued after ALL staging issues (in-order queue).
- PSUM bank = 512 f32 free; 8 banks: f85x2 + f84 + f83 + x + 3 aff.
"""
import numpy as np
from contextlib import ExitStack

import concourse.bass as bass
from concourse import bacc
import concourse.mybir as mybir
import concourse.tile as tile
from concourse.bass_utils import run_bass_kernel_spmd

RAD = 5
W = 56
ROWS = 30            # rows of x per core (26 from + 4 halo)
FROM_ROWS = 26
NPX = ROWS * W       # 1680
NPAIR = FROM_ROWS * 48   # 1248
C = 448
N_CORES = 8

F32 = mybir.dt.float32
BF16 = mybir.dt.bfloat16
I16 = mybir.dt.int16


def _offsets():
    out = []
    for x in range(1, RAD):
        out.append((0, x))
    for y in range(1, RAD):
        for x in range(-RAD + 1, RAD):
            if x * x + y * y < RAD * RAD:
                out.append((y, x))
    return out


OFFS = _offsets()            # 34 (dy, dx), matching reference search_dist order
assert len(OFFS) == 34

# w9 contraction split aligned to feature-group boundaries (f83|f84|f85a|f85b)
KSPLIT = [(0, 64), (64, 128), (192, 128), (320, 128)]
# x output channel groups: 4 groups of <=128 (padded to 128 in storage)
MSPLIT = [(0, 128), (128, 128), (256, 128), (384, 64)]

# pixel slabs (start, width); widths <= 512 (PSUM bank) and sum to NPX
SLABS = [(0, 420), (420, 420), (840, 512), (1352, 328)]
NSLAB = len(SLABS)

# affinity bands: (from_row0, nrows, emit_after_slab_index)
BANDS = [(0, 10, 1), (10, 10, 2), (20, 6, 3)]
for _r0, _nr, _si in BANDS:
    _need = (_r0 + _nr + 4) * W
    _s0, _w = SLABS[_si]
    assert _need <= _s0 + _w, (_r0, _nr, _si)
assert sum(b[1] for b in BANDS) == FROM_ROWS

# per-(band, offset) engine assignment:
# sub: 'v' = DVE, 'p' = Pool (only bands >=1: Pool queue drains staging
# issues first). abs: 'v' = DVE int16 mask, 'a' = ACT Abs.
# pool-dedicated triples: the LAST triples of each band run their subtracts
# on Pool (idle after staging issues) with a dedicated PSUM bank + dt pools
# so the slow pool stream never blocks the DVE-stream rings or queue heads.
POOL_T = [set(), set(), set()]
SUB_E = [['p' if d // 3 in POOL_T[b] else 'v' for d in range(34)]
         for b in range(3)]
_ACT_MOD = [4, 2, 2]   # band0 lighter ACT share (elu overlap during staging)
ABS_E = [['a' if (d % _ACT_MOD[b] == 1 and d // 3 not in POOL_T[b]) else 'v'
          for d in range(34)] for b in range(3)]

# pump schedule: triples pumped per conv section
PUMP_CHUNK = 3    # after each c6 chunk (4 per slab)
PUMP_F84 = 1
PUMP_F83 = 1
PUMP_X = 1        # after each of 4 x m-tiles


def _emit(ctx: ExitStack, tc: "tile.TileContext", io: dict):
    nc = tc.nc
    c6, c5, c4 = io["c6"], io["c5"], io["c4"]
    out_d = io["out"]

    persist = ctx.enter_context(tc.tile_pool(name="persist", bufs=1))
    stage6 = ctx.enter_context(tc.tile_pool(name="stage6", bufs=7))
    stage5 = ctx.enter_context(tc.tile_pool(name="stage5", bufs=2))
    stage4 = ctx.enter_context(tc.tile_pool(name="stage4", bufs=2))
    fpool = ctx.enter_context(tc.tile_pool(name="fpool", bufs=3))
    tpool = ctx.enter_context(tc.tile_pool(name="tmp", bufs=4))
    dpool = ctx.enter_context(tc.tile_pool(name="dtv", bufs=8))
    d2pool = ctx.enter_context(tc.tile_pool(name="dt2", bufs=4))
    pdt = ctx.enter_context(tc.tile_pool(name="pdt", bufs=3))
    pda = ctx.enter_context(tc.tile_pool(name="pda", bufs=2))
    apool = ctx.enter_context(tc.tile_pool(name="aff", bufs=4))
    psF = ctx.enter_context(tc.tile_pool(name="psF", bufs=1, space="PSUM"))
    psX = ctx.enter_context(tc.tile_pool(name="psX", bufs=1, space="PSUM"))
    psA = ctx.enter_context(tc.tile_pool(name="psA", bufs=2, space="PSUM"))
    psP = ctx.enter_context(tc.tile_pool(name="psP", bufs=1, space="PSUM"))

    # ---- weights into SBUF: ONE packed bf16 DMA each ----
    wcs = persist.tile([128, 9472], BF16, name="wcs")
    nc.sync.dma_start(wcs[:], io["wc"][:])
    w9cs = persist.tile([128, 4, 448], BF16, name="w9cs")
    nc.sync.dma_start(w9cs[:], io["w9c"][:].rearrange("p (k m) -> p k m", k=4))

    def w85_sl(kt, m):
        base = kt * 256 + m * 128
        return wcs[:, base:base + 128]

    def w84_sl(kt):
        return wcs[:, 8192 + kt * 128:8192 + (kt + 1) * 128]

    def w83_sl(kt):
        return wcs[:, 9216 + kt * 64:9216 + (kt + 1) * 64]

    ones = persist.tile([128, 1], BF16, name="ones")
    nc.vector.memset(ones[:], 1.0)

    # ---- x storage (bf16, 4x128 padded groups) ----
    xg = persist.tile([128, 4, NPX], BF16, name="xg", tag="xg")
    # zero the pad rows of group 3 (channels 448..511); elu writes 0:64 only
    nc.vector.memset(xg[64:128, 3, :], 0.0)

    # ---- ELU helper: out = max(p, exp(min(p,0)) - 1), p in PSUM.
    def elu(psrc, dst, pn, fn):
        r = tpool.tile([pn, 512], BF16, tag="elu_m", name="elu_m")
        nc.scalar.activation(out=r[:, :fn], in_=psrc, scale=-1.0,
                             func=mybir.ActivationFunctionType.Relu)
        e = tpool.tile([pn, 512], BF16, tag="elu_e", name="elu_e")
        nc.scalar.activation(out=e[:, :fn], in_=r[:, :fn], scale=-1.0,
                             func=mybir.ActivationFunctionType.Exp)
        nc.vector.scalar_tensor_tensor(
            out=dst, in0=e[:, :fn], scalar=-1.0, in1=psrc,
            op0=mybir.AluOpType.add, op1=mybir.AluOpType.max)

    # ---- conv input staging: cast-DMA (fp32 HBM -> bf16 SBUF), 6 DMAs
    # per slab (c5, c4, c6 as 4 chunks of 8 ktiles). A single SWDGE DMA is
    # runtime-split across all 16 queues at the same aggregate bandwidth;
    # issue cost is per-instruction, so fewer+bigger wins.
    v6 = c6[:].rearrange("(k p) n -> p k n", p=128)
    v5 = c5[:].rearrange("(k p) n -> p k n", p=128)
    v4 = c4[:].rearrange("(k p) n -> p k n", p=128)
    cslab = []
    for s0, w in SLABS:
        t5 = stage5.tile([128, 8, 512], BF16, tag="c5", name="c5")
        nc.gpsimd.dma_start(t5[:, :, :w], v5[:, :, s0:s0 + w])
        t4 = stage4.tile([128, 4, 512], BF16, tag="c4", name="c4")
        nc.gpsimd.dma_start(t4[:, :, :w], v4[:, :, s0:s0 + w])
        t6 = []
        for ck in range(4):
            t = stage6.tile([128, 8, 512], BF16, tag="c6", name="c6")
            nc.gpsimd.dma_start(t[:, :, :w], v6[:, ck * 8:(ck + 1) * 8, s0:s0 + w])
            t6.append(t)
        cslab.append({"c6": t6, "c5": t5, "c4": t4})

    xg_r = xg[:].rearrange("p g (r c) -> p g r c", c=W)

    # Band triples are emitted as a generator and pumped between conv
    # sections so band work interleaves into engine queues in dep order.
    def emit_band(bi, r0, nr):
        npair = nr * 48
        for t3 in range(12):
            k = min(3, 34 - t3 * 3)
            is_pool = t3 in POOL_T[bi]
            if is_pool:
                pst = psP.tile([128, 512], F32, tag="pstp", name="pstp")
            else:
                pst = psA.tile([128, 512], F32, tag="pst", name="pst")
            arow = apool.tile([65, 480], F32, tag="arow", name="arow")
            for j in range(k):
                d_idx = t3 * 3 + j
                dy, dx = OFFS[d_idx]
                to_view = xg_r[:, :, r0 + dy:r0 + dy + nr, 4 + dx:52 + dx]
                from_view = xg_r[:, :, r0:r0 + nr, 4:52]
                if is_pool:
                    dt = pdt.tile([128, 4, 10, 48], BF16, tag="pdt", name="pdt")
                else:
                    dt = dpool.tile([128, 4, 10, 48], BF16, tag="dt", name="dt")
                dslice = dt[:, :, 0:nr, :]
                if SUB_E[bi][d_idx] == 'p':
                    nc.gpsimd.tensor_tensor(out=dslice, in0=to_view,
                                            in1=from_view,
                                            op=mybir.AluOpType.subtract)
                else:
                    nc.vector.tensor_tensor(out=dslice, in0=to_view,
                                            in1=from_view,
                                            op=mybir.AluOpType.subtract)
                if is_pool:
                    a = pda.tile([128, 4, 10, 48], BF16, tag="pda", name="pda")
                else:
                    a = d2pool.tile([128, 4, 10, 48], BF16, tag="da", name="da")
                aslice = a[:, :, 0:nr, :]
                if ABS_E[bi][d_idx] == 'a':
                    nc.scalar.activation(out=aslice, in_=dslice,
                                         func=mybir.ActivationFunctionType.Abs)
                else:
                    nc.vector.tensor_scalar(
                        out=aslice.bitcast(I16), in0=dslice.bitcast(I16),
                        scalar1=32767, scalar2=None,
                        op0=mybir.AluOpType.bitwise_and)
                for g in range(4):
                    nc.tensor.matmul(
                        pst[32 * j:32 * j + 1, :npair], ones[:],
                        a[:, g, 0:nr, :],
                        start=(g == 0), stop=(g == 3))
            pn = 32 * (k - 1) + 1
            nc.scalar.activation(out=arow[0:pn, :npair], in_=pst[0:pn, :npair],
                                 func=mybir.ActivationFunctionType.Exp,
                                 scale=-1.0 / C)
            for j in range(k):
                d_out = t3 * 3 + j
                nc.sync.dma_start(
                    out_d[d_out:d_out + 1, r0 * 48:(r0 + nr) * 48],
                    arow[32 * j:32 * j + 1, :npair])
            yield

    band_q = list(BANDS)
    active = []

    def pump(n):
        while n > 0 and active:
            try:
                next(active[0])
                n -= 1
            except StopIteration:
                active.pop(0)

    for s in range(NSLAB):
        s0, w = SLABS[s]
        hs = cslab[s]

        def load6(kt):
            return hs["c6"][kt // 8][:, kt % 8, :w]

        # f85: 2 M-tiles of 128 out-ch
        f85p = [psF.tile([128, 512], F32, tag=f"f85{m}", name=f"f85p{m}")
                for m in range(2)]
        for kt in range(32):
            rhs = load6(kt)
            for m in range(2):
                nc.tensor.matmul(
                    f85p[m][:, :w], w85_sl(kt, m),
                    rhs, start=(kt == 0), stop=(kt == 31))
            if kt % 8 == 7:
                pump(PUMP_CHUNK)
        f84p = psF.tile([128, 512], F32, tag="f84", name="f84p")
        for kt in range(8):
            nc.tensor.matmul(f84p[:, :w], w84_sl(kt), hs["c5"][:, kt, :w],
                             start=(kt == 0), stop=(kt == 7))
        pump(PUMP_F84)
        f83p = psF.tile([64, 512], F32, tag="f83", name="f83p")
        for kt in range(4):
            nc.tensor.matmul(f83p[:, :w], w83_sl(kt), hs["c4"][:, kt, :w],
                             start=(kt == 0), stop=(kt == 3))
        pump(PUMP_F83)

        # ELU f -> sbuf k-group tiles (64/128/128/128 partitions)
        fk = [fpool.tile([kn, 512], BF16, tag=f"fk{i}", name=f"fk{i}")
              for i, (k0, kn) in enumerate(KSPLIT)]
        elu(f83p[:, :w], fk[0][:, :w], 64, w)
        elu(f84p[:, :w], fk[1][:, :w], 128, w)
        elu(f85p[0][:, :w], fk[2][:, :w], 128, w)
        elu(f85p[1][:, :w], fk[3][:, :w], 128, w)

        # x = ELU(w9 @ f): M-tiles sequential to cap PSUM use
        sl = slice(s0, s0 + w)
        for mt, (m0, mn) in enumerate(MSPLIT):
            xp = psX.tile([mn, 512], F32, tag="xp", name="xp")
            for kt in range(4):
                nc.tensor.matmul(xp[:, :w], w9cs[0:KSPLIT[kt][1], kt, m0:m0 + mn],
                                 fk[kt][:, :w], start=(kt == 0), stop=(kt == 3))
            elu(xp[:, :w], xg[0:mn, mt, sl], mn, w)
            pump(PUMP_X)

        # activate any bands whose x pixels are now all written
        while band_q and band_q[0][2] == s:
            r0, nr, _ = band_q.pop(0)
            active.append(emit_band(len(BANDS) - len(band_q) - 1, r0, nr))

    # drain remaining band work
    pump(10 ** 6)


_NC_CACHE = {}
LAST_RESULT = None


def _build_nc():
    if "nc" in _NC_CACHE:
        return _NC_CACHE["nc"]
    nc = bacc.Bacc()
    io = {
        "c6": nc.declare_dram_parameter("c6", [4096, NPX], F32, isOutput=False),
        "c5": nc.declare_dram_parameter("c5", [1024, NPX], F32, isOutput=False),
        "c4": nc.declare_dram_parameter("c4", [512, NPX], F32, isOutput=False),
        "wc": nc.declare_dram_parameter("wc", [128, 9472], BF16, isOutput=False),
        "w9c": nc.declare_dram_parameter("w9c", [128, 4 * 448], BF16, isOutput=False),
        "out": nc.declare_dram_parameter("out", [34, NPAIR], F32, isOutput=True),
    }
    with tile.TileContext(nc) as tc:
        with ExitStack() as ctx:
            _emit(ctx, tc, io)
    nc.finalize()
    _NC_CACHE["nc"] = nc
    return nc


def _expected_indices():
    full = np.reshape(np.arange(0, 56 * 56, dtype=np.int64), (56, 56))
    ind_from = np.reshape(full[:-4, 4:-4], [-1])
    tos = []
    for dy, dx in OFFS:
        tos.append(np.reshape(full[dy:dy + 52, 4 + dx:4 + dx + 48], [-1]))
    return ind_from, np.concatenate(tos, axis=0)


def _maybe_install_trace_hook():
    import os
    if not os.environ.get("BASS_TRACE"):
        return
    import sys
    import types
    try:
        import antenv.axon_hooks  # noqa: F401
        return
    except ImportError:
        pass
    try:
        from trn_agent_boot.trn_boot import _ntff_profile_via_ctypes
        hook = _ntff_profile_via_ctypes('/opt/axon/libaxon_pjrt.so')
    except Exception:
        hook = None
    import antenv
    mod = types.ModuleType("antenv.axon_hooks")
    mod.get_axon_ntff_profile_hook = lambda: hook
    mod.set_axon_ntff_profile_hook = lambda h: None
    sys.modules["antenv.axon_hooks"] = mod
    antenv.axon_hooks = mod


def kernel(conv4, conv5, conv6, w83, w84, w85, w9, ind_from, ind_to):
    import ml_dtypes
    conv4 = np.asarray(conv4, dtype=np.float32)
    conv5 = np.asarray(conv5, dtype=np.float32)
    conv6 = np.asarray(conv6, dtype=np.float32)
    ef, et = _expected_indices()
    assert np.array_equal(np.asarray(ind_from), ef), "unexpected ind_from"
    assert np.array_equal(np.asarray(ind_to), et), "unexpected ind_to"

    def warrange(w, ktiles):
        # w [M, K] -> lhsT tiles layout [128, ktiles*M]
        wt = np.asarray(w, np.float32).T            # [K, M]
        K, M = wt.shape
        return np.ascontiguousarray(
            wt.reshape(ktiles, 128, M).transpose(1, 0, 2).reshape(128, ktiles * M))

    wc = np.ascontiguousarray(np.concatenate(
        [warrange(w85, 32), warrange(w84, 8), warrange(w83, 4)],
        axis=1)).astype(ml_dtypes.bfloat16)
    w9t_f = np.asarray(w9, np.float32).T          # [448 in, 448 out]
    w9c = np.zeros((128, 4, 448), np.float32)
    ks = [(0, 64), (64, 128), (192, 128), (320, 128)]
    for i, (k0, kn) in enumerate(ks):
        w9c[0:kn, i, :] = w9t_f[k0:k0 + kn, :]
    w9c = np.ascontiguousarray(w9c.reshape(128, 4 * 448)).astype(
        ml_dtypes.bfloat16)

    in_maps = []
    for core in range(N_CORES):
        b, half = core // 2, core % 2
        r0 = 0 if half == 0 else 26
        in_maps.append({
            "c6": np.ascontiguousarray(
                conv6[b, :, r0:r0 + ROWS, :].reshape(4096, NPX)),
            "c5": np.ascontiguousarray(
                conv5[b, :, r0:r0 + ROWS, :].reshape(1024, NPX)),
            "c4": np.ascontiguousarray(
                conv4[b, :, r0:r0 + ROWS, :].reshape(512, NPX)),
            "wc": wc, "w9c": w9c,
        })

    _maybe_install_trace_hook()
    nc = _build_nc()
    res = run_bass_kernel_spmd(nc, in_maps, list(range(N_CORES)))
    global LAST_RESULT
    LAST_RESULT = res

    aff = np.empty((4, 34, 2496), np.float32)
    for core in range(N_CORES):
        b, half = core // 2, core % 2
        aff[b, :, half * NPAIR:(half + 1) * NPAIR] = res.results[core]["out"]
    return aff
